# revision 34
# baseline (speedup 1.0000x reference)
"""Expert-parallel grouped GEMM (MoE) kernel for Trainium2.

Problem: out[e] = gelu(tok[e] @ w1[e]) @ w2[e]  per expert e.
  tok: [128, 2048, 128] f32, w1: [128, 128, 512] f32, w2: [128, 512, 128] f32.

Sharding: expert-parallel across 8 NeuronCores, 16 experts per core, no
cross-core communication. Each core runs the same Bass program on its own
expert slice (SPMD), the host concatenates the per-core outputs.

Per-core dataflow (v3):
  - tokens SWDGE-cast f32->bf16 on load, [128 p, 16 m, 128 d] (token t = p*16+m)
  - token transpose to [d, t]: one batched DMA-transpose (X-bar) per expert
    (cfg tok_path="dmat"), or PE transposes + DVE copies (cfg "pe")
  - GEMM1 on PE: w1 bf16 stationary (FWL), tokT moving, N=512 full rate
  - GELU on ACT in groups of `gelu_group` psum banks per instruction
    (amortizes the per-instruction fixed overhead), writes one big bf16
    SBUF tile hsb [128, 4*2048] per expert
  - GEMM2 "direct": stationary = hT 128-token block, moving = w2 tile, psum
    accumulates [t, o] directly -> no output transposes, single DVE drain copy
  - batched per-expert store [128 p, 16 m, 128 o] (HWDGE)
"""

import numpy as np

NUM_CORES = 8
E_TOTAL = 128
E_PER_CORE = E_TOTAL // NUM_CORES  # 16
T = 2048
D = 128
H = 512
O = 128
P = 128

N_BLKS = T // P  # 16 token blocks per expert
N_CHUNKS = 4
BLKS_PER_CHUNK = N_BLKS // N_CHUNKS  # 4
T_CHUNK = T // N_CHUNKS  # 512
H_TILES = H // P  # 4

_CACHE = {}


DEFAULT_CFG = dict(
    # "dmat": batched DMA-transpose; "pe": PE transposes; "host": tokens
    # arrive pre-transposed [E, D, T] with column order t = (m, p), m=t%16
    tok_path="host",
    # "direct": hT stationary, [t,o] psum; "classic": w2 stationary + PE
    # transposes; "dvet": w2 stationary + DVE 32x32 stream-transpose drain,
    # block scatter done by the store DMA access pattern (128B descriptors)
    g2="classic",
    gelu_group=2,  # psum banks per ACT gelu instruction
    # shared_psum: allocate ph/po/pot from ONE psum pool (bufs x 8KB) so the
    # GELU group can be 4 banks; paces the pipeline at ACT rate
    shared_psum=False,
    # popot_shared: po and pot rotate through one 2-slot bank pool (saves 2
    # banks vs separate pools; enables gelu_group=3 together with host tokens)
    popot_shared=True,
    # weave: interleave G2(e-1) chunk bursts between G1(e) psum-group fills so
    # the PE fills its ACT-paced ph-ring wait slots with GEMM2 work (classic only)
    weave=False,
    load_ahead=2,
    ph_bufs=2,
    po_bufs=2,
    pt_bufs=2,
    pot_bufs=2,
    osb_bufs=2,
    tokn_bufs=3,
    tokt_bufs=3,
    h_bufs=2,
    outsb_bufs=3,
    w_bufs=3,
)


def _build(loop=1, cfg=None):
    import concourse.bacc as bacc
    import concourse.mybir as mybir
    import concourse.tile as tile
    from concourse.masks import make_identity

    f32 = mybir.dt.float32
    bf16 = mybir.dt.bfloat16
    GELU = mybir.ActivationFunctionType.Gelu
    C = dict(DEFAULT_CFG)
    if cfg:
        C.update(cfg)

    E = E_PER_CORE
    GG = C["gelu_group"]
    N_TILES = H_TILES * N_CHUNKS  # 16 (hd, c) psum tiles per expert

    from contextlib import ExitStack

    nc = bacc.Bacc(
        "TRN2",
        target_bir_lowering=False,
        debug=False,
        num_devices=NUM_CORES,
    )

    host_t = C["tok_path"] == "host"
    tok_shape = [E, D, T] if host_t else [E, T, D]
    tok = nc.dram_tensor("group_token", tok_shape, f32, kind="ExternalInput").ap()
    w1 = nc.dram_tensor("weights1", [E, D, H], f32, kind="ExternalInput").ap()
    w2 = nc.dram_tensor("weights2", [E, H, O], f32, kind="ExternalInput").ap()
    out = nc.dram_tensor("out", [E, T, O], f32, kind="ExternalOutput").ap()

    with tile.TileContext(nc) as tc:
        with ExitStack() as stack:
            const_pool = stack.enter_context(tc.tile_pool(name="const", bufs=1))
            w_pool = stack.enter_context(tc.tile_pool(name="weights", bufs=C["w_bufs"]))
            tokn_pool = stack.enter_context(tc.tile_pool(name="tokn", bufs=C["tokn_bufs"]))
            tokt_pool = stack.enter_context(tc.tile_pool(name="tokt", bufs=C["tokt_bufs"]))
            h_pool = stack.enter_context(tc.tile_pool(name="hts", bufs=C["h_bufs"]))
            outsb_pool = stack.enter_context(tc.tile_pool(name="outsb", bufs=C["outsb_bufs"]))
            if C["shared_psum"]:
                shared = stack.enter_context(
                    tc.tile_pool(name="psum", bufs=C["ph_bufs"], space="PSUM")
                )
                ph_pool = po_pool = pot_pool_shared = shared
            else:
                ph_pool = stack.enter_context(
                    tc.tile_pool(name="ph", bufs=C["ph_bufs"], space="PSUM")
                )
                po_pool = stack.enter_context(
                    tc.tile_pool(name="po", bufs=C["po_bufs"], space="PSUM")
                )
                pot_pool_shared = None

            need_ident = C["tok_path"] == "pe" or C["g2"] == "classic"
            if need_ident:
                ident_f32 = const_pool.tile([P, P], f32)
                make_identity(nc, ident_f32)
                ident = const_pool.tile([P, P], bf16)
                nc.vector.tensor_copy(ident[:], ident_f32[:])

            if C["tok_path"] == "pe":
                pt_pool = stack.enter_context(
                    tc.tile_pool(name="pt", bufs=C["pt_bufs"], space="PSUM")
                )
            if C["g2"] == "classic":
                if pot_pool_shared is not None:
                    pot_pool = pot_pool_shared
                elif C["popot_shared"]:
                    pot_pool = po_pool
                else:
                    pot_pool = stack.enter_context(
                        tc.tile_pool(name="pot", bufs=C["pot_bufs"], space="PSUM")
                    )
                osb_pool = stack.enter_context(
                    tc.tile_pool(name="osb", bufs=C["osb_bufs"])
                )

            def body(_iv=None):
                tokn = {}
                tokT = {}
                hsb = {}

                def load(e):
                    if host_t:
                        # tokens already [D, T] with col order (m, p); cast-load
                        # straight into the transposed SBUF layout
                        tt = tokt_pool.tile(
                            [P, N_BLKS, P], bf16, tag="tokt", name=f"tokt{e}"
                        )
                        nc.gpsimd.dma_start(
                            tt[:], tok[e].rearrange("d (m p) -> d m p", p=P)
                        )
                        tokT[e] = tt
                    else:
                        tkn = tokn_pool.tile(
                            [P, N_BLKS, D], bf16, tag="tokn", name=f"tokn{e}"
                        )
                        nc.gpsimd.dma_start(
                            tkn[:], tok[e].rearrange("(p m) d -> p m d", p=P)
                        )
                        tokn[e] = tkn
                    w1bf = w_pool.tile([P, H], bf16, tag="w1", name=f"w1b{e}")
                    nc.gpsimd.dma_start(w1bf[:], w1[e])
                    w2bf = w_pool.tile([P, H_TILES, O], bf16, tag="w2", name=f"w2b{e}")
                    nc.gpsimd.dma_start(
                        w2bf[:], w2[e].rearrange("(k p) o -> p k o", p=P)
                    )
                    tokn[e, "w"] = (w1bf, w2bf)

                def tin(e):
                    if host_t:
                        return
                    # tokT[d, m, p]: token t = p*16 + m lives at column m*128+p
                    tt = tokt_pool.tile([P, N_BLKS, P], bf16, tag="tokt", name=f"tokt{e}")
                    if C["tok_path"] == "dmat":
                        nc.sync.dma_start(
                            tt[:],
                            tokn[e][:].rearrange("p m d -> p (m d)"),
                            transpose=True,
                        )
                    else:
                        for c in range(N_CHUNKS):
                            pt = pt_pool.tile([P, T_CHUNK], bf16, tag="pt")
                            for j in range(BLKS_PER_CHUNK):
                                nc.tensor.transpose(
                                    pt[:, j * P : (j + 1) * P],
                                    tokn[e][:, c * BLKS_PER_CHUNK + j],
                                    ident[:],
                                )
                            nc.vector.tensor_copy(
                                tt[:, c * BLKS_PER_CHUNK : (c + 1) * BLKS_PER_CHUNK],
                                pt[:].rearrange("p (m q) -> p m q", m=BLKS_PER_CHUNK),
                            )
                    tokT[e] = tt

                def g1_units(e):
                    """One unit per (ph group fill + gelu) for weaving."""
                    w1bf, _ = tokn[e, "w"]
                    # hsb columns: flat = (hd*4 + c)*512 + i
                    hs = h_pool.tile([P, H_TILES * T], bf16, tag="hsb", name=f"hsb{e}")
                    hsb[e] = hs
                    tt = tokT[e][:].rearrange("p m q -> p (m q)")
                    units = []
                    flat = 0
                    while flat < N_TILES:
                        gsz = min(GG, N_TILES - flat)

                        def unit(base=flat, gsz=gsz):
                            ph = ph_pool.tile(
                                [P, gsz, T_CHUNK],
                                f32,
                                tag="ps" if C["shared_psum"] else "ph",
                                padded_shape=[P, GG, T_CHUNK],
                            )
                            for i in range(gsz):
                                hd, c = divmod(base + i, N_CHUNKS)
                                nc.tensor.matmul(
                                    ph[:, i],
                                    w1bf[:, hd * P : (hd + 1) * P],
                                    tt[:, c * T_CHUNK : (c + 1) * T_CHUNK],
                                    start=True,
                                    stop=True,
                                )
                            nc.scalar.activation(
                                hs[:, base * T_CHUNK : (base + gsz) * T_CHUNK],
                                ph[:].rearrange("p g q -> p (g q)"),
                                GELU,
                            )

                        units.append(unit)
                        flat += gsz
                    return units

                def g1(e):
                    for unit in g1_units(e):
                        unit()

                def g2_direct(e):
                    _, w2bf = tokn[e, "w"]
                    hs = hsb[e]
                    osb = outsb_pool.tile([P, N_BLKS, O], f32, tag="outsb", name=f"osb{e}")
                    for c in range(N_CHUNKS):
                        po = po_pool.tile(
                            [P, BLKS_PER_CHUNK, O],
                            f32,
                            tag="ps" if C["shared_psum"] else "po",
                        )
                        for j in range(BLKS_PER_CHUNK):
                            m = c * BLKS_PER_CHUNK + j
                            for hd in range(H_TILES):
                                nc.tensor.matmul(
                                    po[:, j],
                                    hs[:, (hd * N_BLKS + m) * P : (hd * N_BLKS + m + 1) * P],
                                    w2bf[:, hd],
                                    start=(hd == 0),
                                    stop=(hd == H_TILES - 1),
                                )
                        nc.vector.tensor_copy(
                            osb[:, c * BLKS_PER_CHUNK : (c + 1) * BLKS_PER_CHUNK],
                            po[:],
                        )
                    return osb

                def g2_classic(e):
                    _, w2bf = tokn[e, "w"]
                    hs = hsb[e]
                    osb_out = outsb_pool.tile(
                        [P, N_BLKS, O], f32, tag="outsb", name=f"osb{e}"
                    )
                    if C["shared_psum"]:
                        po_tag = pot_tag = "ps"
                    elif C["popot_shared"]:
                        po_tag = pot_tag = "popot"
                    else:
                        po_tag, pot_tag = "po", "pot"
                    obs = {}

                    def mm_chunk(c):
                        po = po_pool.tile([P, T_CHUNK], f32, tag=po_tag)
                        for hd in range(H_TILES):
                            nc.tensor.matmul(
                                po[:],
                                w2bf[:, hd],
                                hs[:, (hd * N_CHUNKS + c) * T_CHUNK : (hd * N_CHUNKS + c + 1) * T_CHUNK],
                                start=(hd == 0),
                                stop=(hd == H_TILES - 1),
                            )
                        ob = osb_pool.tile([P, T_CHUNK], bf16, tag="ob")
                        nc.vector.tensor_copy(ob[:], po[:])
                        obs[c] = ob

                    def tout_chunk(c):
                        pot = pot_pool.tile([P, T_CHUNK], bf16, tag=pot_tag)
                        for j in range(BLKS_PER_CHUNK):
                            nc.tensor.transpose(
                                pot[:, j * P : (j + 1) * P],
                                obs[c][:, j * P : (j + 1) * P],
                                ident[:],
                            )
                        nc.vector.tensor_copy(
                            osb_out[:, c * BLKS_PER_CHUNK : (c + 1) * BLKS_PER_CHUNK],
                            pot[:].rearrange("p (m q) -> p m q", m=BLKS_PER_CHUNK),
                        )

                    # software-pipelined: Tout(c) emitted after MMs(c+1) so the
                    # PE never waits on the DVE psum->sbuf copy of chunk c
                    def unit(c):
                        if c < N_CHUNKS:
                            mm_chunk(c)
                        if c > 0:
                            tout_chunk(c - 1)

                    for c in range(N_CHUNKS + 1):
                        unit(c)
                    return osb_out

                def g2_classic_units(e):
                    """Same as g2_classic but yields per-chunk units for
                    weaving between g1 group fills."""
                    _, w2bf = tokn[e, "w"]
                    hs = hsb[e]
                    osb_out = outsb_pool.tile(
                        [P, N_BLKS, O], f32, tag="outsb", name=f"osb{e}"
                    )
                    if C["shared_psum"]:
                        po_tag = pot_tag = "ps"
                    elif C["popot_shared"]:
                        po_tag = pot_tag = "popot"
                    else:
                        po_tag, pot_tag = "po", "pot"
                    obs = {}

                    def mm_chunk(c):
                        po = po_pool.tile([P, T_CHUNK], f32, tag=po_tag)
                        for hd in range(H_TILES):
                            nc.tensor.matmul(
                                po[:],
                                w2bf[:, hd],
                                hs[:, (hd * N_CHUNKS + c) * T_CHUNK : (hd * N_CHUNKS + c + 1) * T_CHUNK],
                                start=(hd == 0),
                                stop=(hd == H_TILES - 1),
                            )
                        ob = osb_pool.tile([P, T_CHUNK], bf16, tag="ob")
                        nc.vector.tensor_copy(ob[:], po[:])
                        obs[c] = ob

                    def tout_chunk(c):
                        pot = pot_pool.tile([P, T_CHUNK], bf16, tag=pot_tag)
                        for j in range(BLKS_PER_CHUNK):
                            nc.tensor.transpose(
                                pot[:, j * P : (j + 1) * P],
                                obs[c][:, j * P : (j + 1) * P],
                                ident[:],
                            )
                        nc.vector.tensor_copy(
                            osb_out[:, c * BLKS_PER_CHUNK : (c + 1) * BLKS_PER_CHUNK],
                            pot[:].rearrange("p (m q) -> p m q", m=BLKS_PER_CHUNK),
                        )

                    units = []
                    for c in range(N_CHUNKS + 1):

                        def unit(c=c):
                            if c < N_CHUNKS:
                                mm_chunk(c)
                            if c > 0:
                                tout_chunk(c - 1)

                        units.append(unit)
                    return osb_out, units

                def g2_dvet(e):
                    # po[o, q] (q = within-chunk col) -> DVE 32x32 block
                    # transpose -> osb_out[:, c] holds X[p=(g,i), (hm,hp,j)]
                    # where o = 32g+j, token col q = 128hm + 32hp + i
                    _, w2bf = tokn[e, "w"]
                    hs = hsb[e]
                    osb_out = outsb_pool.tile(
                        [P, N_CHUNKS, T_CHUNK], f32, tag="outsb", name=f"osb{e}"
                    )
                    for c in range(N_CHUNKS):
                        po = po_pool.tile([P, T_CHUNK], f32, tag="po")
                        for hd in range(H_TILES):
                            nc.tensor.matmul(
                                po[:],
                                w2bf[:, hd],
                                hs[:, (hd * N_CHUNKS + c) * T_CHUNK : (hd * N_CHUNKS + c + 1) * T_CHUNK],
                                start=(hd == 0),
                                stop=(hd == H_TILES - 1),
                            )
                        nc.vector.transpose(osb_out[:, c], po[:])
                    return osb_out

                if C["g2"] == "direct":
                    g2 = g2_direct
                elif C["g2"] == "dvet":
                    g2 = g2_dvet
                else:
                    g2 = g2_classic

                def store(e, osb):
                    if C["g2"] == "dvet":
                        # token t = p_tok*16 + 4c + hm, p_tok = 32*hp + i,
                        # o = 32g + j; sbuf free order (c, hm, hp, j)
                        out_re = out[e].rearrange(
                            "(hp i c hm) (g j) -> hp g i c hm j",
                            hp=4,
                            i=32,
                            c=N_CHUNKS,
                            hm=4,
                            g=4,
                        )
                        osb_re = osb[:].rearrange(
                            "p c (hm hp j) -> hp p c hm j", hm=4, hp=4
                        )
                        for hp in range(4):
                            nc.sync.dma_start(out_re[hp], osb_re[hp])
                    else:
                        nc.sync.dma_start(
                            out[e].rearrange("(p m) o -> p m o", p=P), osb[:]
                        )

                LA = C["load_ahead"]
                for e in range(min(LA, E)):
                    load(e)
                tin(0)
                if C["weave"] and C["g2"] == "classic":
                    for e in range(E):
                        if e + LA < E:
                            load(e + LA)
                        if e + 1 < E:
                            tin(e + 1)
                        g1u = g1_units(e)
                        if e > 0:
                            osb_prev, g2u = g2_classic_units(e - 1)
                        else:
                            osb_prev, g2u = None, []
                        # lead with 2 g1 fills, then round-robin
                        sched = []
                        gi = g2i = 0
                        while gi < len(g1u) or g2i < len(g2u):
                            take_g1 = 2 if gi == 0 else 1
                            for _ in range(take_g1):
                                if gi < len(g1u):
                                    sched.append(g1u[gi])
                                    gi += 1
                            if g2i < len(g2u):
                                sched.append(g2u[g2i])
                                g2i += 1
                        for unit in sched:
                            unit()
                        if osb_prev is not None:
                            store(e - 1, osb_prev)
                    osb_last, g2u = g2_classic_units(E - 1)
                    for unit in g2u:
                        unit()
                    store(E - 1, osb_last)
                else:
                    pending = {}
                    for e in range(E):
                        if e + LA < E:
                            load(e + LA)
                        if e + 1 < E:
                            tin(e + 1)
                        g1(e)
                        if e > 0:
                            pending[e - 1] = g2(e - 1)
                            store(e - 1, pending.pop(e - 1))
                    pending[E - 1] = g2(E - 1)
                    store(E - 1, pending.pop(E - 1))

            if loop == 1:
                body()
            else:
                with tc.For_i(0, loop, 1) as _i:
                    body(_i)

    nc.compile()
    return nc


def _get_nc(loop=1, cfg=None):
    key = ("nc", loop, tuple(sorted((cfg or {}).items())))
    if key not in _CACHE:
        _CACHE[key] = _build(loop, cfg)
    return _CACHE[key]


ACTIVE_CFG = None  # overrides DEFAULT_CFG for kernel() when set


def host_transpose_tokens(tok_slice):
    """[E, T, D] -> [E, D, T] with column order t = (m, p), token t = p*16+m."""
    E = tok_slice.shape[0]
    return np.ascontiguousarray(
        tok_slice.reshape(E, P, N_BLKS, D).transpose(0, 3, 2, 1).reshape(E, D, T)
    )


def make_in_maps(group_token, weights1, weights2, cfg=None):
    C = dict(DEFAULT_CFG)
    if cfg:
        C.update(cfg)
    host_t = C["tok_path"] == "host"
    in_maps = []
    for c in range(NUM_CORES):
        sl = slice(c * E_PER_CORE, (c + 1) * E_PER_CORE)
        tok_c = group_token[sl]
        tok_c = (
            host_transpose_tokens(tok_c)
            if host_t
            else np.ascontiguousarray(tok_c)
        )
        in_maps.append(
            {
                "group_token": tok_c,
                "weights1": np.ascontiguousarray(weights1[sl]),
                "weights2": np.ascontiguousarray(weights2[sl]),
            }
        )
    return in_maps


def kernel(group_token, weights1, weights2):
    from concourse.bass_utils import run_bass_kernel_spmd

    group_token = np.asarray(group_token, dtype=np.float32)
    weights1 = np.asarray(weights1, dtype=np.float32)
    weights2 = np.asarray(weights2, dtype=np.float32)

    nc = _get_nc(cfg=ACTIVE_CFG)
    in_maps = make_in_maps(group_token, weights1, weights2, ACTIVE_CFG)

    res = run_bass_kernel_spmd(nc, in_maps, core_ids=list(range(NUM_CORES)))
    _CACHE["last_results"] = res
    return np.concatenate([r["out"] for r in res.results], axis=0)


# revision 36
# speedup vs baseline: 1.0530x; 1.0530x over previous
"""Expert-parallel grouped GEMM (MoE) kernel for Trainium2.

Problem: out[e] = gelu(tok[e] @ w1[e]) @ w2[e]  per expert e.
  tok: [128, 2048, 128] f32, w1: [128, 128, 512] f32, w2: [128, 512, 128] f32.

Sharding: expert-parallel across 8 NeuronCores, 16 experts per core, no
cross-core communication. Each core runs the same Bass program on its own
expert slice (SPMD), the host concatenates the per-core outputs.

Per-core dataflow (v3):
  - tokens SWDGE-cast f32->bf16 on load, [128 p, 16 m, 128 d] (token t = p*16+m)
  - token transpose to [d, t]: one batched DMA-transpose (X-bar) per expert
    (cfg tok_path="dmat"), or PE transposes + DVE copies (cfg "pe")
  - GEMM1 on PE: w1 bf16 stationary (FWL), tokT moving, N=512 full rate
  - GELU on ACT in groups of `gelu_group` psum banks per instruction
    (amortizes the per-instruction fixed overhead), writes one big bf16
    SBUF tile hsb [128, 4*2048] per expert
  - GEMM2 "direct": stationary = hT 128-token block, moving = w2 tile, psum
    accumulates [t, o] directly -> no output transposes, single DVE drain copy
  - batched per-expert store [128 p, 16 m, 128 o] (HWDGE)
"""

import numpy as np

NUM_CORES = 8
E_TOTAL = 128
E_PER_CORE = E_TOTAL // NUM_CORES  # 16
T = 2048
D = 128
H = 512
O = 128
P = 128

N_BLKS = T // P  # 16 token blocks per expert
N_CHUNKS = 4
BLKS_PER_CHUNK = N_BLKS // N_CHUNKS  # 4
T_CHUNK = T // N_CHUNKS  # 512
H_TILES = H // P  # 4

_CACHE = {}


DEFAULT_CFG = dict(
    # "dmat": batched DMA-transpose; "pe": PE transposes; "host": tokens
    # arrive pre-transposed [E, D, T] with column order t = (m, p), m=t%16
    tok_path="host",
    # "direct": hT stationary, [t,o] psum; "classic": w2 stationary + PE
    # transposes; "dvet": w2 stationary + DVE 32x32 stream-transpose drain,
    # block scatter done by the store DMA access pattern (128B descriptors)
    g2="classic",
    gelu_group=2,  # psum fp32 banks per ACT gelu instruction
    # shared_psum: allocate ph/po/pot from ONE psum pool (bufs x 8KB) so the
    # GELU group can be 4 banks; paces the pipeline at ACT rate
    shared_psum=False,
    # popot_shared: po and pot rotate through one 2-slot bank pool (saves 2
    # banks vs separate pools; enables gelu_group=3 together with host tokens)
    popot_shared=True,
    # weave: interleave G2(e-1) chunk bursts between G1(e) psum-group fills so
    # the PE fills its ACT-paced ph-ring wait slots with GEMM2 work (classic only)
    weave=False,
    load_ahead=3,
    ph_bufs=2,
    po_bufs=2,
    pt_bufs=2,
    pot_bufs=2,
    osb_bufs=2,
    tokn_bufs=3,
    tokt_bufs=4,
    h_bufs=3,
    outsb_bufs=3,
    w_bufs=4,
)


def _build(loop=1, cfg=None):
    import concourse.bacc as bacc
    import concourse.mybir as mybir
    import concourse.tile as tile
    from concourse.masks import make_identity

    f32 = mybir.dt.float32
    bf16 = mybir.dt.bfloat16
    GELU = mybir.ActivationFunctionType.Gelu
    C = dict(DEFAULT_CFG)
    if cfg:
        C.update(cfg)

    E = E_PER_CORE
    GG = C["gelu_group"]
    N_TILES = H_TILES * N_CHUNKS  # 16 (hd, c) psum tiles per expert

    from contextlib import ExitStack

    nc = bacc.Bacc(
        "TRN2",
        target_bir_lowering=False,
        debug=False,
        num_devices=NUM_CORES,
    )

    host_t = C["tok_path"] == "host"
    tok_shape = [E, D, T] if host_t else [E, T, D]
    tok = nc.dram_tensor("group_token", tok_shape, f32, kind="ExternalInput").ap()
    w1 = nc.dram_tensor("weights1", [E, D, H], f32, kind="ExternalInput").ap()
    w2 = nc.dram_tensor("weights2", [E, H, O], f32, kind="ExternalInput").ap()
    out = nc.dram_tensor("out", [E, T, O], f32, kind="ExternalOutput").ap()

    with tile.TileContext(nc) as tc:
        with ExitStack() as stack:
            const_pool = stack.enter_context(tc.tile_pool(name="const", bufs=1))
            w_pool = stack.enter_context(tc.tile_pool(name="weights", bufs=C["w_bufs"]))
            tokn_pool = stack.enter_context(tc.tile_pool(name="tokn", bufs=C["tokn_bufs"]))
            tokt_pool = stack.enter_context(tc.tile_pool(name="tokt", bufs=C["tokt_bufs"]))
            h_pool = stack.enter_context(tc.tile_pool(name="hts", bufs=C["h_bufs"]))
            outsb_pool = stack.enter_context(tc.tile_pool(name="outsb", bufs=C["outsb_bufs"]))
            if C["shared_psum"]:
                shared = stack.enter_context(
                    tc.tile_pool(name="psum", bufs=C["ph_bufs"], space="PSUM")
                )
                ph_pool = po_pool = pot_pool_shared = shared
            else:
                ph_pool = stack.enter_context(
                    tc.tile_pool(name="ph", bufs=C["ph_bufs"], space="PSUM")
                )
                po_pool = stack.enter_context(
                    tc.tile_pool(name="po", bufs=C["po_bufs"], space="PSUM")
                )
                pot_pool_shared = None

            need_ident = C["tok_path"] == "pe" or C["g2"] == "classic"
            if need_ident:
                ident_f32 = const_pool.tile([P, P], f32)
                make_identity(nc, ident_f32)
                ident = const_pool.tile([P, P], bf16)
                nc.vector.tensor_copy(ident[:], ident_f32[:])

            if C["tok_path"] == "pe":
                pt_pool = stack.enter_context(
                    tc.tile_pool(name="pt", bufs=C["pt_bufs"], space="PSUM")
                )
            if C["g2"] == "classic":
                if pot_pool_shared is not None:
                    pot_pool = pot_pool_shared
                elif C["popot_shared"]:
                    pot_pool = po_pool
                else:
                    pot_pool = stack.enter_context(
                        tc.tile_pool(name="pot", bufs=C["pot_bufs"], space="PSUM")
                    )
                osb_pool = stack.enter_context(
                    tc.tile_pool(name="osb", bufs=C["osb_bufs"])
                )

            def body(_iv=None):
                tokn = {}
                tokT = {}
                hsb = {}

                def load(e):
                    if host_t:
                        # tokens already [D, T] with col order (m, p); cast-load
                        # straight into the transposed SBUF layout
                        tt = tokt_pool.tile(
                            [P, N_BLKS, P], bf16, tag="tokt", name=f"tokt{e}"
                        )
                        nc.gpsimd.dma_start(
                            tt[:], tok[e].rearrange("d (m p) -> d m p", p=P)
                        )
                        tokT[e] = tt
                    else:
                        tkn = tokn_pool.tile(
                            [P, N_BLKS, D], bf16, tag="tokn", name=f"tokn{e}"
                        )
                        nc.gpsimd.dma_start(
                            tkn[:], tok[e].rearrange("(p m) d -> p m d", p=P)
                        )
                        tokn[e] = tkn
                    w1bf = w_pool.tile([P, H], bf16, tag="w1", name=f"w1b{e}")
                    nc.gpsimd.dma_start(w1bf[:], w1[e])
                    w2bf = w_pool.tile([P, H_TILES, O], bf16, tag="w2", name=f"w2b{e}")
                    nc.gpsimd.dma_start(
                        w2bf[:], w2[e].rearrange("(k p) o -> p k o", p=P)
                    )
                    tokn[e, "w"] = (w1bf, w2bf)

                def tin(e):
                    if host_t:
                        return
                    # tokT[d, m, p]: token t = p*16 + m lives at column m*128+p
                    tt = tokt_pool.tile([P, N_BLKS, P], bf16, tag="tokt", name=f"tokt{e}")
                    if C["tok_path"] == "dmat":
                        nc.sync.dma_start(
                            tt[:],
                            tokn[e][:].rearrange("p m d -> p (m d)"),
                            transpose=True,
                        )
                    else:
                        for c in range(N_CHUNKS):
                            pt = pt_pool.tile([P, T_CHUNK], bf16, tag="pt")
                            for j in range(BLKS_PER_CHUNK):
                                nc.tensor.transpose(
                                    pt[:, j * P : (j + 1) * P],
                                    tokn[e][:, c * BLKS_PER_CHUNK + j],
                                    ident[:],
                                )
                            nc.vector.tensor_copy(
                                tt[:, c * BLKS_PER_CHUNK : (c + 1) * BLKS_PER_CHUNK],
                                pt[:].rearrange("p (m q) -> p m q", m=BLKS_PER_CHUNK),
                            )
                    tokT[e] = tt

                def g1_units(e):
                    """One unit per (ph group fill + gelu) for weaving."""
                    w1bf, _ = tokn[e, "w"]
                    # hsb columns: flat = (hd*4 + c)*512 + i
                    hs = h_pool.tile([P, H_TILES * T], bf16, tag="hsb", name=f"hsb{e}")
                    hsb[e] = hs
                    tt = tokT[e][:].rearrange("p m q -> p (m q)")
                    units = []
                    flat = 0
                    while flat < N_TILES:
                        gsz = min(GG, N_TILES - flat)

                        def unit(base=flat, gsz=gsz):
                            ph = ph_pool.tile(
                                [P, gsz, T_CHUNK],
                                f32,
                                tag="ps" if C["shared_psum"] else "ph",
                                padded_shape=[P, GG, T_CHUNK],
                            )
                            for i in range(gsz):
                                hd, c = divmod(base + i, N_CHUNKS)
                                nc.tensor.matmul(
                                    ph[:, i],
                                    w1bf[:, hd * P : (hd + 1) * P],
                                    tt[:, c * T_CHUNK : (c + 1) * T_CHUNK],
                                    start=True,
                                    stop=True,
                                )
                            nc.scalar.activation(
                                hs[:, base * T_CHUNK : (base + gsz) * T_CHUNK],
                                ph[:].rearrange("p g q -> p (g q)"),
                                GELU,
                            )

                        units.append(unit)
                        flat += gsz
                    return units

                def g1(e):
                    for unit in g1_units(e):
                        unit()

                def g2_direct(e):
                    _, w2bf = tokn[e, "w"]
                    hs = hsb[e]
                    osb = outsb_pool.tile([P, N_BLKS, O], f32, tag="outsb", name=f"osb{e}")
                    for c in range(N_CHUNKS):
                        po = po_pool.tile(
                            [P, BLKS_PER_CHUNK, O],
                            f32,
                            tag="ps" if C["shared_psum"] else "po",
                        )
                        for j in range(BLKS_PER_CHUNK):
                            m = c * BLKS_PER_CHUNK + j
                            for hd in range(H_TILES):
                                nc.tensor.matmul(
                                    po[:, j],
                                    hs[:, (hd * N_BLKS + m) * P : (hd * N_BLKS + m + 1) * P],
                                    w2bf[:, hd],
                                    start=(hd == 0),
                                    stop=(hd == H_TILES - 1),
                                )
                        nc.vector.tensor_copy(
                            osb[:, c * BLKS_PER_CHUNK : (c + 1) * BLKS_PER_CHUNK],
                            po[:],
                        )
                    return osb

                def g2_classic(e):
                    _, w2bf = tokn[e, "w"]
                    hs = hsb[e]
                    osb_out = outsb_pool.tile(
                        [P, N_BLKS, O], f32, tag="outsb", name=f"osb{e}"
                    )
                    if C["shared_psum"]:
                        po_tag = pot_tag = "ps"
                    elif C["popot_shared"]:
                        po_tag = pot_tag = "popot"
                    else:
                        po_tag, pot_tag = "po", "pot"
                    obs = {}

                    def mm_chunk(c):
                        po = po_pool.tile([P, T_CHUNK], f32, tag=po_tag)
                        for hd in range(H_TILES):
                            nc.tensor.matmul(
                                po[:],
                                w2bf[:, hd],
                                hs[:, (hd * N_CHUNKS + c) * T_CHUNK : (hd * N_CHUNKS + c + 1) * T_CHUNK],
                                start=(hd == 0),
                                stop=(hd == H_TILES - 1),
                            )
                        ob = osb_pool.tile([P, T_CHUNK], bf16, tag="ob")
                        nc.vector.tensor_copy(ob[:], po[:])
                        obs[c] = ob

                    def tout_chunk(c):
                        pot = pot_pool.tile([P, T_CHUNK], bf16, tag=pot_tag)
                        for j in range(BLKS_PER_CHUNK):
                            nc.tensor.transpose(
                                pot[:, j * P : (j + 1) * P],
                                obs[c][:, j * P : (j + 1) * P],
                                ident[:],
                            )
                        nc.vector.tensor_copy(
                            osb_out[:, c * BLKS_PER_CHUNK : (c + 1) * BLKS_PER_CHUNK],
                            pot[:].rearrange("p (m q) -> p m q", m=BLKS_PER_CHUNK),
                        )

                    # software-pipelined: Tout(c) emitted after MMs(c+1) so the
                    # PE never waits on the DVE psum->sbuf copy of chunk c
                    def unit(c):
                        if c < N_CHUNKS:
                            mm_chunk(c)
                        if c > 0:
                            tout_chunk(c - 1)

                    for c in range(N_CHUNKS + 1):
                        unit(c)
                    return osb_out

                def g2_classic_units(e):
                    """Same as g2_classic but yields per-chunk units for
                    weaving between g1 group fills."""
                    _, w2bf = tokn[e, "w"]
                    hs = hsb[e]
                    osb_out = outsb_pool.tile(
                        [P, N_BLKS, O], f32, tag="outsb", name=f"osb{e}"
                    )
                    if C["shared_psum"]:
                        po_tag = pot_tag = "ps"
                    elif C["popot_shared"]:
                        po_tag = pot_tag = "popot"
                    else:
                        po_tag, pot_tag = "po", "pot"
                    obs = {}

                    def mm_chunk(c):
                        po = po_pool.tile([P, T_CHUNK], f32, tag=po_tag)
                        for hd in range(H_TILES):
                            nc.tensor.matmul(
                                po[:],
                                w2bf[:, hd],
                                hs[:, (hd * N_CHUNKS + c) * T_CHUNK : (hd * N_CHUNKS + c + 1) * T_CHUNK],
                                start=(hd == 0),
                                stop=(hd == H_TILES - 1),
                            )
                        ob = osb_pool.tile([P, T_CHUNK], bf16, tag="ob")
                        nc.vector.tensor_copy(ob[:], po[:])
                        obs[c] = ob

                    def tout_chunk(c):
                        pot = pot_pool.tile([P, T_CHUNK], bf16, tag=pot_tag)
                        for j in range(BLKS_PER_CHUNK):
                            nc.tensor.transpose(
                                pot[:, j * P : (j + 1) * P],
                                obs[c][:, j * P : (j + 1) * P],
                                ident[:],
                            )
                        nc.vector.tensor_copy(
                            osb_out[:, c * BLKS_PER_CHUNK : (c + 1) * BLKS_PER_CHUNK],
                            pot[:].rearrange("p (m q) -> p m q", m=BLKS_PER_CHUNK),
                        )

                    units = []
                    for c in range(N_CHUNKS + 1):

                        def unit(c=c):
                            if c < N_CHUNKS:
                                mm_chunk(c)
                            if c > 0:
                                tout_chunk(c - 1)

                        units.append(unit)
                    return osb_out, units

                def g2_dvet(e):
                    # po[o, q] (q = within-chunk col) -> DVE 32x32 block
                    # transpose -> osb_out[:, c] holds X[p=(g,i), (hm,hp,j)]
                    # where o = 32g+j, token col q = 128hm + 32hp + i
                    _, w2bf = tokn[e, "w"]
                    hs = hsb[e]
                    osb_out = outsb_pool.tile(
                        [P, N_CHUNKS, T_CHUNK], f32, tag="outsb", name=f"osb{e}"
                    )
                    for c in range(N_CHUNKS):
                        po = po_pool.tile([P, T_CHUNK], f32, tag="po")
                        for hd in range(H_TILES):
                            nc.tensor.matmul(
                                po[:],
                                w2bf[:, hd],
                                hs[:, (hd * N_CHUNKS + c) * T_CHUNK : (hd * N_CHUNKS + c + 1) * T_CHUNK],
                                start=(hd == 0),
                                stop=(hd == H_TILES - 1),
                            )
                        nc.vector.transpose(osb_out[:, c], po[:])
                    return osb_out

                if C["g2"] == "direct":
                    g2 = g2_direct
                elif C["g2"] == "dvet":
                    g2 = g2_dvet
                else:
                    g2 = g2_classic

                def store(e, osb):
                    if C["g2"] == "dvet":
                        # token t = p_tok*16 + 4c + hm, p_tok = 32*hp + i,
                        # o = 32g + j; sbuf free order (c, hm, hp, j)
                        out_re = out[e].rearrange(
                            "(hp i c hm) (g j) -> hp g i c hm j",
                            hp=4,
                            i=32,
                            c=N_CHUNKS,
                            hm=4,
                            g=4,
                        )
                        osb_re = osb[:].rearrange(
                            "p c (hm hp j) -> hp p c hm j", hm=4, hp=4
                        )
                        for hp in range(4):
                            nc.sync.dma_start(out_re[hp], osb_re[hp])
                    else:
                        nc.sync.dma_start(
                            out[e].rearrange("(p m) o -> p m o", p=P), osb[:]
                        )

                LA = C["load_ahead"]
                for e in range(min(LA, E)):
                    load(e)
                tin(0)
                if C["weave"] and C["g2"] == "classic":
                    for e in range(E):
                        if e + LA < E:
                            load(e + LA)
                        if e + 1 < E:
                            tin(e + 1)
                        g1u = g1_units(e)
                        if e > 0:
                            osb_prev, g2u = g2_classic_units(e - 1)
                        else:
                            osb_prev, g2u = None, []
                        # lead with 2 g1 fills, then round-robin
                        sched = []
                        gi = g2i = 0
                        while gi < len(g1u) or g2i < len(g2u):
                            take_g1 = 2 if gi == 0 else 1
                            for _ in range(take_g1):
                                if gi < len(g1u):
                                    sched.append(g1u[gi])
                                    gi += 1
                            if g2i < len(g2u):
                                sched.append(g2u[g2i])
                                g2i += 1
                        for unit in sched:
                            unit()
                        if osb_prev is not None:
                            store(e - 1, osb_prev)
                    osb_last, g2u = g2_classic_units(E - 1)
                    for unit in g2u:
                        unit()
                    store(E - 1, osb_last)
                else:
                    pending = {}
                    for e in range(E):
                        if e + LA < E:
                            load(e + LA)
                        if e + 1 < E:
                            tin(e + 1)
                        g1(e)
                        if e > 0:
                            pending[e - 1] = g2(e - 1)
                            store(e - 1, pending.pop(e - 1))
                    pending[E - 1] = g2(E - 1)
                    store(E - 1, pending.pop(E - 1))

            if loop == 1:
                body()
            else:
                with tc.For_i(0, loop, 1) as _i:
                    body(_i)

    nc.compile()
    return nc


def _get_nc(loop=1, cfg=None):
    key = ("nc", loop, tuple(sorted((cfg or {}).items())))
    if key not in _CACHE:
        _CACHE[key] = _build(loop, cfg)
    return _CACHE[key]


ACTIVE_CFG = None  # overrides DEFAULT_CFG for kernel() when set


def host_transpose_tokens(tok_slice):
    """[E, T, D] -> [E, D, T] with column order t = (m, p), token t = p*16+m."""
    E = tok_slice.shape[0]
    return np.ascontiguousarray(
        tok_slice.reshape(E, P, N_BLKS, D).transpose(0, 3, 2, 1).reshape(E, D, T)
    )


def make_in_maps(group_token, weights1, weights2, cfg=None):
    C = dict(DEFAULT_CFG)
    if cfg:
        C.update(cfg)
    host_t = C["tok_path"] == "host"
    in_maps = []
    for c in range(NUM_CORES):
        sl = slice(c * E_PER_CORE, (c + 1) * E_PER_CORE)
        tok_c = group_token[sl]
        tok_c = (
            host_transpose_tokens(tok_c)
            if host_t
            else np.ascontiguousarray(tok_c)
        )
        in_maps.append(
            {
                "group_token": tok_c,
                "weights1": np.ascontiguousarray(weights1[sl]),
                "weights2": np.ascontiguousarray(weights2[sl]),
            }
        )
    return in_maps


def kernel(group_token, weights1, weights2):
    from concourse.bass_utils import run_bass_kernel_spmd

    group_token = np.asarray(group_token, dtype=np.float32)
    weights1 = np.asarray(weights1, dtype=np.float32)
    weights2 = np.asarray(weights2, dtype=np.float32)

    nc = _get_nc(cfg=ACTIVE_CFG)
    in_maps = make_in_maps(group_token, weights1, weights2, ACTIVE_CFG)

    res = run_bass_kernel_spmd(nc, in_maps, core_ids=list(range(NUM_CORES)))
    _CACHE["last_results"] = res
    return np.concatenate([r["out"] for r in res.results], axis=0)


# revision 39
# speedup vs baseline: 1.0875x; 1.0329x over previous
"""Expert-parallel grouped GEMM (MoE) kernel for Trainium2.

Problem: out[e] = gelu(tok[e] @ w1[e]) @ w2[e]  per expert e.
  tok: [128, 2048, 128] f32, w1: [128, 128, 512] f32, w2: [128, 512, 128] f32.

Sharding: expert-parallel across 8 NeuronCores, 16 experts per core, no
cross-core communication. Each core runs the same Bass program on its own
expert slice (SPMD), the host concatenates the per-core outputs.

Per-core dataflow (v3):
  - tokens SWDGE-cast f32->bf16 on load, [128 p, 16 m, 128 d] (token t = p*16+m)
  - token transpose to [d, t]: one batched DMA-transpose (X-bar) per expert
    (cfg tok_path="dmat"), or PE transposes + DVE copies (cfg "pe")
  - GEMM1 on PE: w1 bf16 stationary (FWL), tokT moving, N=512 full rate
  - GELU on ACT in groups of `gelu_group` psum banks per instruction
    (amortizes the per-instruction fixed overhead), writes one big bf16
    SBUF tile hsb [128, 4*2048] per expert
  - GEMM2 "direct": stationary = hT 128-token block, moving = w2 tile, psum
    accumulates [t, o] directly -> no output transposes, single DVE drain copy
  - batched per-expert store [128 p, 16 m, 128 o] (HWDGE)
"""

import numpy as np

NUM_CORES = 8
E_TOTAL = 128
E_PER_CORE = E_TOTAL // NUM_CORES  # 16
T = 2048
D = 128
H = 512
O = 128
P = 128

N_BLKS = T // P  # 16 token blocks per expert
N_CHUNKS = 4
BLKS_PER_CHUNK = N_BLKS // N_CHUNKS  # 4
T_CHUNK = T // N_CHUNKS  # 512
H_TILES = H // P  # 4

_CACHE = {}


DEFAULT_CFG = dict(
    # "dmat": batched DMA-transpose; "pe": PE transposes; "host": tokens
    # arrive pre-transposed [E, D, T] with column order t = (m, p), m=t%16
    tok_path="host",
    # "direct": hT stationary, [t,o] psum; "classic": w2 stationary + PE
    # transposes; "dvet": w2 stationary + DVE 32x32 stream-transpose drain,
    # block scatter done by the store DMA access pattern (128B descriptors)
    g2="classic",
    gelu_group=2,  # psum fp32 banks per ACT gelu instruction
    gelu_sched=None,  # optional explicit group-size list summing to 16, e.g. (3,3,3,3,2,2)
    # shared_psum: allocate ph/po/pot from ONE psum pool (bufs x 8KB) so the
    # GELU group can be 4 banks; paces the pipeline at ACT rate
    shared_psum=False,
    # popot_shared: po and pot rotate through one 2-slot bank pool (saves 2
    # banks vs separate pools; enables gelu_group=3 together with host tokens)
    popot_shared=True,
    # weave: interleave G2(e-1) chunk bursts between G1(e) psum-group fills so
    # the PE fills its ACT-paced ph-ring wait slots with GEMM2 work (classic only)
    weave=False,
    load_ahead=3,
    ph_bufs=2,
    po_bufs=2,
    pt_bufs=2,
    pot_bufs=2,
    osb_bufs=2,
    tokn_bufs=3,
    tokt_bufs=4,
    h_bufs=3,
    outsb_bufs=3,
    w_bufs=4,
)


def _build(loop=1, cfg=None):
    import concourse.bacc as bacc
    import concourse.mybir as mybir
    import concourse.tile as tile
    from concourse.masks import make_identity

    f32 = mybir.dt.float32
    bf16 = mybir.dt.bfloat16
    GELU = mybir.ActivationFunctionType.Gelu
    C = dict(DEFAULT_CFG)
    if cfg:
        C.update(cfg)

    E = E_PER_CORE
    GG = C["gelu_group"]
    N_TILES = H_TILES * N_CHUNKS  # 16 (hd, c) psum tiles per expert

    from contextlib import ExitStack

    nc = bacc.Bacc(
        "TRN2",
        target_bir_lowering=False,
        debug=False,
        num_devices=NUM_CORES,
    )

    host_t = C["tok_path"] == "host"
    tok_shape = [E, D, T] if host_t else [E, T, D]
    tok = nc.dram_tensor("group_token", tok_shape, f32, kind="ExternalInput").ap()
    w1 = nc.dram_tensor("weights1", [E, D, H], f32, kind="ExternalInput").ap()
    w2 = nc.dram_tensor("weights2", [E, H, O], f32, kind="ExternalInput").ap()
    out = nc.dram_tensor("out", [E, T, O], f32, kind="ExternalOutput").ap()

    with tile.TileContext(nc) as tc:
        with ExitStack() as stack:
            const_pool = stack.enter_context(tc.tile_pool(name="const", bufs=1))
            w_pool = stack.enter_context(tc.tile_pool(name="weights", bufs=C["w_bufs"]))
            tokn_pool = stack.enter_context(tc.tile_pool(name="tokn", bufs=C["tokn_bufs"]))
            tokt_pool = stack.enter_context(tc.tile_pool(name="tokt", bufs=C["tokt_bufs"]))
            h_pool = stack.enter_context(tc.tile_pool(name="hts", bufs=C["h_bufs"]))
            outsb_pool = stack.enter_context(tc.tile_pool(name="outsb", bufs=C["outsb_bufs"]))
            if C["shared_psum"]:
                shared = stack.enter_context(
                    tc.tile_pool(name="psum", bufs=C["ph_bufs"], space="PSUM")
                )
                ph_pool = po_pool = pot_pool_shared = shared
            else:
                ph_pool = stack.enter_context(
                    tc.tile_pool(name="ph", bufs=C["ph_bufs"], space="PSUM")
                )
                po_pool = stack.enter_context(
                    tc.tile_pool(name="po", bufs=C["po_bufs"], space="PSUM")
                )
                pot_pool_shared = None

            need_ident = C["tok_path"] == "pe" or C["g2"] == "classic"
            if need_ident:
                ident_f32 = const_pool.tile([P, P], f32)
                make_identity(nc, ident_f32)
                ident = const_pool.tile([P, P], bf16)
                nc.vector.tensor_copy(ident[:], ident_f32[:])

            if C["tok_path"] == "pe":
                pt_pool = stack.enter_context(
                    tc.tile_pool(name="pt", bufs=C["pt_bufs"], space="PSUM")
                )
            if C["g2"] == "classic":
                if pot_pool_shared is not None:
                    pot_pool = pot_pool_shared
                elif C["popot_shared"]:
                    pot_pool = po_pool
                else:
                    pot_pool = stack.enter_context(
                        tc.tile_pool(name="pot", bufs=C["pot_bufs"], space="PSUM")
                    )
                osb_pool = stack.enter_context(
                    tc.tile_pool(name="osb", bufs=C["osb_bufs"])
                )

            def body(_iv=None):
                tokn = {}
                tokT = {}
                hsb = {}

                def load(e):
                    if host_t:
                        # tokens already [D, T] with col order (m, p); cast-load
                        # straight into the transposed SBUF layout
                        tt = tokt_pool.tile(
                            [P, N_BLKS, P], bf16, tag="tokt", name=f"tokt{e}"
                        )
                        nc.gpsimd.dma_start(
                            tt[:], tok[e].rearrange("d (m p) -> d m p", p=P)
                        )
                        tokT[e] = tt
                    else:
                        tkn = tokn_pool.tile(
                            [P, N_BLKS, D], bf16, tag="tokn", name=f"tokn{e}"
                        )
                        nc.gpsimd.dma_start(
                            tkn[:], tok[e].rearrange("(p m) d -> p m d", p=P)
                        )
                        tokn[e] = tkn
                    w1bf = w_pool.tile([P, H], bf16, tag="w1", name=f"w1b{e}")
                    nc.gpsimd.dma_start(w1bf[:], w1[e])
                    w2bf = w_pool.tile([P, H_TILES, O], bf16, tag="w2", name=f"w2b{e}")
                    nc.gpsimd.dma_start(
                        w2bf[:], w2[e].rearrange("(k p) o -> p k o", p=P)
                    )
                    tokn[e, "w"] = (w1bf, w2bf)

                def tin(e):
                    if host_t:
                        return
                    # tokT[d, m, p]: token t = p*16 + m lives at column m*128+p
                    tt = tokt_pool.tile([P, N_BLKS, P], bf16, tag="tokt", name=f"tokt{e}")
                    if C["tok_path"] == "dmat":
                        nc.sync.dma_start(
                            tt[:],
                            tokn[e][:].rearrange("p m d -> p (m d)"),
                            transpose=True,
                        )
                    else:
                        for c in range(N_CHUNKS):
                            pt = pt_pool.tile([P, T_CHUNK], bf16, tag="pt")
                            for j in range(BLKS_PER_CHUNK):
                                nc.tensor.transpose(
                                    pt[:, j * P : (j + 1) * P],
                                    tokn[e][:, c * BLKS_PER_CHUNK + j],
                                    ident[:],
                                )
                            nc.vector.tensor_copy(
                                tt[:, c * BLKS_PER_CHUNK : (c + 1) * BLKS_PER_CHUNK],
                                pt[:].rearrange("p (m q) -> p m q", m=BLKS_PER_CHUNK),
                            )
                    tokT[e] = tt

                def g1_units(e):
                    """One unit per (ph group fill + gelu) for weaving."""
                    w1bf, _ = tokn[e, "w"]
                    # hsb columns: flat = (hd*4 + c)*512 + i
                    hs = h_pool.tile([P, H_TILES * T], bf16, tag="hsb", name=f"hsb{e}")
                    hsb[e] = hs
                    tt = tokT[e][:].rearrange("p m q -> p (m q)")
                    if C["gelu_sched"]:
                        sched = list(C["gelu_sched"])
                        assert sum(sched) == N_TILES
                        max_g = max(sched)
                    else:
                        sched = None
                        max_g = GG
                    units = []
                    flat = 0
                    while flat < N_TILES:
                        gsz = (
                            sched[len(units)] if sched else min(GG, N_TILES - flat)
                        )

                        def unit(base=flat, gsz=gsz):
                            ph = ph_pool.tile(
                                [P, gsz, T_CHUNK],
                                f32,
                                tag="ps" if C["shared_psum"] else "ph",
                                padded_shape=[P, max_g, T_CHUNK],
                            )
                            for i in range(gsz):
                                hd, c = divmod(base + i, N_CHUNKS)
                                nc.tensor.matmul(
                                    ph[:, i],
                                    w1bf[:, hd * P : (hd + 1) * P],
                                    tt[:, c * T_CHUNK : (c + 1) * T_CHUNK],
                                    start=True,
                                    stop=True,
                                )
                            nc.scalar.activation(
                                hs[:, base * T_CHUNK : (base + gsz) * T_CHUNK],
                                ph[:].rearrange("p g q -> p (g q)"),
                                GELU,
                            )

                        units.append(unit)
                        flat += gsz
                    return units

                def g1(e):
                    for unit in g1_units(e):
                        unit()

                def g2_direct(e):
                    _, w2bf = tokn[e, "w"]
                    hs = hsb[e]
                    osb = outsb_pool.tile([P, N_BLKS, O], f32, tag="outsb", name=f"osb{e}")
                    for c in range(N_CHUNKS):
                        po = po_pool.tile(
                            [P, BLKS_PER_CHUNK, O],
                            f32,
                            tag="ps" if C["shared_psum"] else "po",
                        )
                        for j in range(BLKS_PER_CHUNK):
                            m = c * BLKS_PER_CHUNK + j
                            for hd in range(H_TILES):
                                nc.tensor.matmul(
                                    po[:, j],
                                    hs[:, (hd * N_BLKS + m) * P : (hd * N_BLKS + m + 1) * P],
                                    w2bf[:, hd],
                                    start=(hd == 0),
                                    stop=(hd == H_TILES - 1),
                                )
                        nc.vector.tensor_copy(
                            osb[:, c * BLKS_PER_CHUNK : (c + 1) * BLKS_PER_CHUNK],
                            po[:],
                        )
                    return osb

                def g2_classic(e):
                    _, w2bf = tokn[e, "w"]
                    hs = hsb[e]
                    osb_out = outsb_pool.tile(
                        [P, N_BLKS, O], f32, tag="outsb", name=f"osb{e}"
                    )
                    if C["shared_psum"]:
                        po_tag = pot_tag = "ps"
                    elif C["popot_shared"]:
                        po_tag = pot_tag = "popot"
                    else:
                        po_tag, pot_tag = "po", "pot"
                    obs = {}

                    def mm_chunk(c):
                        po = po_pool.tile([P, T_CHUNK], f32, tag=po_tag)
                        for hd in range(H_TILES):
                            nc.tensor.matmul(
                                po[:],
                                w2bf[:, hd],
                                hs[:, (hd * N_CHUNKS + c) * T_CHUNK : (hd * N_CHUNKS + c + 1) * T_CHUNK],
                                start=(hd == 0),
                                stop=(hd == H_TILES - 1),
                            )
                        ob = osb_pool.tile([P, T_CHUNK], bf16, tag="ob")
                        nc.vector.tensor_copy(ob[:], po[:])
                        obs[c] = ob

                    def tout_chunk(c):
                        pot = pot_pool.tile([P, T_CHUNK], bf16, tag=pot_tag)
                        for j in range(BLKS_PER_CHUNK):
                            nc.tensor.transpose(
                                pot[:, j * P : (j + 1) * P],
                                obs[c][:, j * P : (j + 1) * P],
                                ident[:],
                            )
                        nc.vector.tensor_copy(
                            osb_out[:, c * BLKS_PER_CHUNK : (c + 1) * BLKS_PER_CHUNK],
                            pot[:].rearrange("p (m q) -> p m q", m=BLKS_PER_CHUNK),
                        )

                    # software-pipelined: Tout(c) emitted after MMs(c+1) so the
                    # PE never waits on the DVE psum->sbuf copy of chunk c
                    def unit(c):
                        if c < N_CHUNKS:
                            mm_chunk(c)
                        if c > 0:
                            tout_chunk(c - 1)

                    for c in range(N_CHUNKS + 1):
                        unit(c)
                    return osb_out

                def g2_classic_units(e):
                    """Same as g2_classic but yields per-chunk units for
                    weaving between g1 group fills."""
                    _, w2bf = tokn[e, "w"]
                    hs = hsb[e]
                    osb_out = outsb_pool.tile(
                        [P, N_BLKS, O], f32, tag="outsb", name=f"osb{e}"
                    )
                    if C["shared_psum"]:
                        po_tag = pot_tag = "ps"
                    elif C["popot_shared"]:
                        po_tag = pot_tag = "popot"
                    else:
                        po_tag, pot_tag = "po", "pot"
                    obs = {}

                    def mm_chunk(c):
                        po = po_pool.tile([P, T_CHUNK], f32, tag=po_tag)
                        for hd in range(H_TILES):
                            nc.tensor.matmul(
                                po[:],
                                w2bf[:, hd],
                                hs[:, (hd * N_CHUNKS + c) * T_CHUNK : (hd * N_CHUNKS + c + 1) * T_CHUNK],
                                start=(hd == 0),
                                stop=(hd == H_TILES - 1),
                            )
                        ob = osb_pool.tile([P, T_CHUNK], bf16, tag="ob")
                        nc.vector.tensor_copy(ob[:], po[:])
                        obs[c] = ob

                    def tout_chunk(c):
                        pot = pot_pool.tile([P, T_CHUNK], bf16, tag=pot_tag)
                        for j in range(BLKS_PER_CHUNK):
                            nc.tensor.transpose(
                                pot[:, j * P : (j + 1) * P],
                                obs[c][:, j * P : (j + 1) * P],
                                ident[:],
                            )
                        nc.vector.tensor_copy(
                            osb_out[:, c * BLKS_PER_CHUNK : (c + 1) * BLKS_PER_CHUNK],
                            pot[:].rearrange("p (m q) -> p m q", m=BLKS_PER_CHUNK),
                        )

                    units = []
                    for c in range(N_CHUNKS + 1):

                        def unit(c=c):
                            if c < N_CHUNKS:
                                mm_chunk(c)
                            if c > 0:
                                tout_chunk(c - 1)

                        units.append(unit)
                    return osb_out, units

                def g2_dvet(e):
                    # po[o, q] (q = within-chunk col) -> DVE 32x32 block
                    # transpose -> osb_out[:, c] holds X[p=(g,i), (hm,hp,j)]
                    # where o = 32g+j, token col q = 128hm + 32hp + i
                    _, w2bf = tokn[e, "w"]
                    hs = hsb[e]
                    osb_out = outsb_pool.tile(
                        [P, N_CHUNKS, T_CHUNK], f32, tag="outsb", name=f"osb{e}"
                    )
                    for c in range(N_CHUNKS):
                        po = po_pool.tile([P, T_CHUNK], f32, tag="po")
                        for hd in range(H_TILES):
                            nc.tensor.matmul(
                                po[:],
                                w2bf[:, hd],
                                hs[:, (hd * N_CHUNKS + c) * T_CHUNK : (hd * N_CHUNKS + c + 1) * T_CHUNK],
                                start=(hd == 0),
                                stop=(hd == H_TILES - 1),
                            )
                        nc.vector.transpose(osb_out[:, c], po[:])
                    return osb_out

                if C["g2"] == "direct":
                    g2 = g2_direct
                elif C["g2"] == "dvet":
                    g2 = g2_dvet
                else:
                    g2 = g2_classic

                def store(e, osb):
                    if C["g2"] == "dvet":
                        # token t = p_tok*16 + 4c + hm, p_tok = 32*hp + i,
                        # o = 32g + j; sbuf free order (c, hm, hp, j)
                        out_re = out[e].rearrange(
                            "(hp i c hm) (g j) -> hp g i c hm j",
                            hp=4,
                            i=32,
                            c=N_CHUNKS,
                            hm=4,
                            g=4,
                        )
                        osb_re = osb[:].rearrange(
                            "p c (hm hp j) -> hp p c hm j", hm=4, hp=4
                        )
                        for hp in range(4):
                            nc.sync.dma_start(out_re[hp], osb_re[hp])
                    else:
                        nc.sync.dma_start(
                            out[e].rearrange("(p m) o -> p m o", p=P), osb[:]
                        )

                LA = C["load_ahead"]
                for e in range(min(LA, E)):
                    load(e)
                tin(0)
                if C["weave"] and C["g2"] == "classic":
                    for e in range(E):
                        if e + LA < E:
                            load(e + LA)
                        if e + 1 < E:
                            tin(e + 1)
                        g1u = g1_units(e)
                        if e > 0:
                            osb_prev, g2u = g2_classic_units(e - 1)
                        else:
                            osb_prev, g2u = None, []
                        # lead with 2 g1 fills, then round-robin
                        sched = []
                        gi = g2i = 0
                        while gi < len(g1u) or g2i < len(g2u):
                            take_g1 = 2 if gi == 0 else 1
                            for _ in range(take_g1):
                                if gi < len(g1u):
                                    sched.append(g1u[gi])
                                    gi += 1
                            if g2i < len(g2u):
                                sched.append(g2u[g2i])
                                g2i += 1
                        for unit in sched:
                            unit()
                        if osb_prev is not None:
                            store(e - 1, osb_prev)
                    osb_last, g2u = g2_classic_units(E - 1)
                    for unit in g2u:
                        unit()
                    store(E - 1, osb_last)
                else:
                    pending = {}
                    for e in range(E):
                        if e + LA < E:
                            load(e + LA)
                        if e + 1 < E:
                            tin(e + 1)
                        g1(e)
                        if e > 0:
                            pending[e - 1] = g2(e - 1)
                            store(e - 1, pending.pop(e - 1))
                    pending[E - 1] = g2(E - 1)
                    store(E - 1, pending.pop(E - 1))

            if loop == 1:
                body()
            else:
                with tc.For_i(0, loop, 1) as _i:
                    body(_i)

    nc.compile()
    return nc


def _get_nc(loop=1, cfg=None):
    key = ("nc", loop, tuple(sorted((cfg or {}).items())))
    if key not in _CACHE:
        _CACHE[key] = _build(loop, cfg)
    return _CACHE[key]


ACTIVE_CFG = None  # overrides DEFAULT_CFG for kernel() when set


def host_transpose_tokens(tok_slice):
    """[E, T, D] -> [E, D, T] with column order t = (m, p), token t = p*16+m."""
    E = tok_slice.shape[0]
    return np.ascontiguousarray(
        tok_slice.reshape(E, P, N_BLKS, D).transpose(0, 3, 2, 1).reshape(E, D, T)
    )


def make_in_maps(group_token, weights1, weights2, cfg=None):
    C = dict(DEFAULT_CFG)
    if cfg:
        C.update(cfg)
    host_t = C["tok_path"] == "host"
    in_maps = []
    for c in range(NUM_CORES):
        sl = slice(c * E_PER_CORE, (c + 1) * E_PER_CORE)
        tok_c = group_token[sl]
        tok_c = (
            host_transpose_tokens(tok_c)
            if host_t
            else np.ascontiguousarray(tok_c)
        )
        in_maps.append(
            {
                "group_token": tok_c,
                "weights1": np.ascontiguousarray(weights1[sl]),
                "weights2": np.ascontiguousarray(weights2[sl]),
            }
        )
    return in_maps


def kernel(group_token, weights1, weights2):
    from concourse.bass_utils import run_bass_kernel_spmd

    group_token = np.asarray(group_token, dtype=np.float32)
    weights1 = np.asarray(weights1, dtype=np.float32)
    weights2 = np.asarray(weights2, dtype=np.float32)

    nc = _get_nc(cfg=ACTIVE_CFG)
    in_maps = make_in_maps(group_token, weights1, weights2, ACTIVE_CFG)

    res = run_bass_kernel_spmd(nc, in_maps, core_ids=list(range(NUM_CORES)))
    _CACHE["last_results"] = res
    return np.concatenate([r["out"] for r in res.results], axis=0)
